# revision 1
# baseline (speedup 1.0000x reference)
"""Trainium2 Bass kernel for BackgroundNoiseLayer (gnn_message_passing).

Computation (reference semantics):
    vals[e, r] = weights[e] * tau_syn[e, r]
    W[n, k, r] = scatter_add(vals over (rows, cols))        # [N, K, R]
    out[b, n, r] = sum_k W[n, k, r] * spikes[b, k]          # [BT, N, R]
    return out.reshape(1, BT, N*R)

Sharding: neuron dim N=50000 split across 8 cores (6250 rows each).
spikes is replicated; each core computes its [BT, 6250*R] output slice
fully locally; host concatenates the slices.

Scatter strategy: rows/cols are structure (fixed at model init), so all
index math happens on the host at kernel-build time. Each core receives
dense zero-padded "round" arrays laid out exactly as the matmul wants W:
    w_j[k, n]   = weight of the j-th edge in bucket (n,k), else 0
    tau_j[k, n*5+r] = tau of that edge, else 0
Device computes W = w_0*tau_0 (+ w_1*tau_1 via a second accumulating
matmul pass) entirely in f32, and the einsum as out = spikesT.T @ W on
the PE. Buckets with >=3 edges (~110 per core of 62.5k edges) are merged
into round 1 on the host (w_1=1, tau_1 = sum_{j>=1} w_j*tau_j).
"""

import numpy as np

import concourse.bass as bass
import concourse.tile as tile
from concourse import bacc, mybir
from concourse.bass_utils import run_bass_kernel_spmd

N_NEURONS = 50000
N_BKG = 100          # K (contraction dim)
R = 5                # synapse basis
BT = 250             # batch*time
N_CORES = 8
NLOC = N_NEURONS // N_CORES       # 6250 rows per core
NR = NLOC * R                     # 31250 free-dim elements per core
BH = BT // 2                      # 125, output partition half

F32 = mybir.dt.float32
F32R = mybir.dt.float32r

# nr-chunks: 12 x 2500 + 1 x 1250 (chunk width divisible by R=5)
CHUNKS = [(i * 2500, 2500) for i in range(12)] + [(30000, 1250)]


def _mm_tiles(width):
    """Split a chunk width into matmul free-dim tiles (<=500, f32 PSUM bank)."""
    tiles = []
    off = 0
    while off < width:
        tw = min(500, width - off)
        tiles.append((off, tw))
        off += tw
    return tiles


def _build_program(use_f32r=True):
    wdt = F32R if use_f32r else F32
    nc = bacc.Bacc("TRN2", target_bir_lowering=False, debug=False,
                   num_devices=N_CORES)

    w0_d = nc.dram_tensor("w0", [N_BKG, NLOC], F32, kind="ExternalInput").ap()
    tau0_d = nc.dram_tensor("tau0", [N_BKG, NR], F32, kind="ExternalInput").ap()
    w1_d = nc.dram_tensor("w1", [N_BKG, NLOC], F32, kind="ExternalInput").ap()
    tau1_d = nc.dram_tensor("tau1", [N_BKG, NR], F32, kind="ExternalInput").ap()
    spikesT_d = nc.dram_tensor("spikesT", [N_BKG, BT], F32,
                               kind="ExternalInput").ap()
    out_d = nc.dram_tensor("out", [BT, NR], F32, kind="ExternalOutput").ap()

    with tile.TileContext(nc) as tc:
        with (
            tc.tile_pool(name="const", bufs=1) as const,
            tc.tile_pool(name="win", bufs=3) as win,
            tc.tile_pool(name="tauin", bufs=3) as tauin,
            tc.tile_pool(name="wbuild", bufs=3) as wbuild,
            tc.tile_pool(name="psum", bufs=6, space="PSUM") as psum,
            tc.tile_pool(name="stage", bufs=4) as stage,
        ):
            st_f32 = const.tile([N_BKG, BT], F32, tag="st_raw")
            nc.sync.dma_start(st_f32[:], spikesT_d[:])
            st = const.tile([N_BKG, BT], wdt, tag="st")
            nc.vector.tensor_copy(st[:], st_f32[:])

            for (s, wd) in CHUNKS:
                nw = wd // R
                ns = s // R
                w0c = win.tile([N_BKG, nw], F32, tag="w0c")
                nc.sync.dma_start(w0c[:], w0_d[:, ns:ns + nw])
                tau0c = tauin.tile([N_BKG, wd], F32, tag="tau0c")
                nc.sync.dma_start(tau0c[:], tau0_d[:, s:s + wd])
                w1c = win.tile([N_BKG, nw], F32, tag="w1c")
                nc.sync.dma_start(w1c[:], w1_d[:, ns:ns + nw])
                tau1c = tauin.tile([N_BKG, wd], F32, tag="tau1c")
                nc.sync.dma_start(tau1c[:], tau1_d[:, s:s + wd])

                W0c = wbuild.tile([N_BKG, wd], wdt, tag="W0c")
                nc.vector.tensor_mul(
                    W0c[:].rearrange("k (n r) -> k n r", r=R),
                    w0c[:].unsqueeze(2).broadcast_to([N_BKG, nw, R]),
                    tau0c[:].rearrange("k (n r) -> k n r", r=R),
                )
                W1c = wbuild.tile([N_BKG, wd], wdt, tag="W1c")
                nc.vector.tensor_mul(
                    W1c[:].rearrange("k (n r) -> k n r", r=R),
                    w1c[:].unsqueeze(2).broadcast_to([N_BKG, nw, R]),
                    tau1c[:].rearrange("k (n r) -> k n r", r=R),
                )

                for h in range(2):
                    st_h = st[:, h * BH:(h + 1) * BH]
                    stg = stage.tile([BH, wd], F32, tag="stage")
                    for (t0, tw) in _mm_tiles(wd):
                        ps = psum.tile([BH, tw], F32, tag="ps")
                        nc.tensor.matmul(ps[:], st_h, W0c[:, t0:t0 + tw],
                                         start=True, stop=False)
                        nc.tensor.matmul(ps[:], st_h, W1c[:, t0:t0 + tw],
                                         start=False, stop=True)
                        nc.any.tensor_copy(stg[:, t0:t0 + tw], ps[:])
                    nc.sync.dma_start(
                        out_d[h * BH:(h + 1) * BH, s:s + wd], stg[:])

    nc.compile()
    return nc


def _preprocess(weights, tau_syn, rows, cols):
    """Host index preprocessing: build per-core dense round arrays."""
    E = rows.shape[0]
    rows = rows.astype(np.int64)
    cols = cols.astype(np.int64)
    core = rows // NLOC
    nloc = rows % NLOC

    # bucket identity and slot (occurrence index) per edge
    q = rows * N_BKG + cols
    order = np.argsort(q, kind="stable")
    qs = q[order]
    first = np.r_[True, qs[1:] != qs[:-1]]
    run_id = np.cumsum(first) - 1
    run_starts = np.flatnonzero(first)
    run_len = np.diff(np.r_[run_starts, E])
    slot = np.empty(E, np.int64)
    slot[order] = np.arange(E) - run_starts[run_id]
    cnt = np.empty(E, np.int64)
    cnt[order] = run_len[run_id]

    # dense layouts: w [k, n] flat = k*NLOC + n ; tau [k, n*R+r]
    wflat = cols * NLOC + nloc
    tauflat = wflat * R  # == k*NR + n*R

    w0d = np.zeros((N_CORES, N_BKG * NLOC), np.float32)
    tau0d = np.zeros((N_CORES, N_BKG * NR), np.float32)
    w1d = np.zeros((N_CORES, N_BKG * NLOC), np.float32)
    tau1d = np.zeros((N_CORES, N_BKG * NR), np.float32)

    roff = np.arange(R)

    m0 = slot == 0
    w0d[core[m0], wflat[m0]] = weights[m0]
    tau0d[core[m0][:, None], tauflat[m0][:, None] + roff] = tau_syn[m0]

    # round 1, buckets with exactly 2 edges: plain (w, tau)
    m1s = (slot == 1) & (cnt == 2)
    w1d[core[m1s], wflat[m1s]] = weights[m1s]
    tau1d[core[m1s][:, None], tauflat[m1s][:, None] + roff] = tau_syn[m1s]

    # buckets with >=3 edges (~0.2% of edges): merge slots>=1 on host
    # (w1=1, tau1 = sum_j>=1 w_j*tau_j)
    mm = (slot >= 1) & (cnt >= 3)
    if mm.any():
        m1m = (slot == 1) & (cnt >= 3)
        w1d[core[m1m], wflat[m1m]] = 1.0
        contrib = weights[mm, None] * tau_syn[mm]
        np.add.at(tau1d, (core[mm][:, None], tauflat[mm][:, None] + roff),
                  contrib)

    return (w0d.reshape(N_CORES, N_BKG, NLOC),
            tau0d.reshape(N_CORES, N_BKG, NR),
            w1d.reshape(N_CORES, N_BKG, NLOC),
            tau1d.reshape(N_CORES, N_BKG, NR))


_program_cache = {}


def get_program(use_f32r=True):
    key = bool(use_f32r)
    if key not in _program_cache:
        _program_cache[key] = _build_program(use_f32r)
    return _program_cache[key]


def make_in_maps(weights, tau_syn, spikes, rows, cols):
    weights = np.ascontiguousarray(np.asarray(weights, dtype=np.float32))
    tau_syn = np.ascontiguousarray(np.asarray(tau_syn, dtype=np.float32))
    spikes = np.ascontiguousarray(np.asarray(spikes, dtype=np.float32))
    rows = np.asarray(rows)
    cols = np.asarray(cols)

    w0d, tau0d, w1d, tau1d = _preprocess(weights, tau_syn, rows, cols)
    spikesT = np.ascontiguousarray(spikes.T)

    in_maps = []
    for c in range(N_CORES):
        in_maps.append({
            "w0": np.ascontiguousarray(w0d[c]),
            "tau0": np.ascontiguousarray(tau0d[c]),
            "w1": np.ascontiguousarray(w1d[c]),
            "tau1": np.ascontiguousarray(tau1d[c]),
            "spikesT": spikesT,
        })
    return in_maps


def kernel(weights, tau_syn, spikes, rows, cols):
    nc = get_program(use_f32r=True)
    in_maps = make_in_maps(weights, tau_syn, spikes, rows, cols)
    res = run_bass_kernel_spmd(nc, in_maps, list(range(N_CORES)))
    full = np.concatenate(
        [res.results[c]["out"] for c in range(N_CORES)], axis=1)
    return full.reshape(1, BT, N_NEURONS * R)


# revision 19
# speedup vs baseline: 243.0109x; 243.0109x over previous
"""Trainium2 Bass kernel for BackgroundNoiseLayer (gnn_message_passing).

Computation (reference semantics):
    vals[e, r] = weights[e] * tau_syn[e, r]
    W[n, k, r] = scatter_add(vals over (rows, cols))        # [N, K, R]
    out[b, n, r] = sum_k W[n, k, r] * spikes[b, k]          # [BT, N, R]
    return out.reshape(1, BT, N*R)

Sharding: neuron dim N=50000 split across 8 cores (6250 rows each).
spikes is replicated; each core computes its [BT, 6250*R] output slice
fully locally; host concatenates the slices.

Scatter strategy: rows/cols are structure (fixed at model init), so all
index math happens on the host at kernel-build time. Each core receives
dense zero-padded "round" arrays laid out exactly as the matmul wants W:
    w_j[k, n]       = weight of the j-th edge in bucket (n,k), else 0
    tau_j[k, n*5+r] = tau of that edge (int16 fixed point), else 0
Device computes W0 = w_0*tau_0 and W1 = w_1*tau_1 on DVE (f32 x int16 ->
f32r), and the einsum as two accumulating PE passes per PSUM tile:
out = spikesT.T @ W0 + spikesT.T @ W1. Buckets with >=3 edges (~110 per
core of 62.5k edges) are merged into round 1 on the host with a
per-bucket power-of-2 scale so the int16 encoding stays exact-ish.

tau quantization: tau in [0,1) is sent as round(tau*32768) clipped to
32767 (error ~3e-5, far below the f32r matmul rounding of ~1.2e-4) and
the matching w is pre-divided by 32768 (exact power-of-2 scaling).
This halves the input read bytes, which is the binding DMA constraint.

DMA layout: outputs (32MB) on Sync HWDGE; tau0 on Scalar HWDGE split as
64+36 partitions (a 128/64-partition HWDGE DMA spreads over 16 engines,
100 partitions only 10); tau1/w/spikes on GPSIMD SWDGE. Output
partitions are padded to 128 per half (out rows 125-127/253-255 are
zeros from zero spike columns) because 128-partition HWDGE DMAs spread
across all 16 SDMA engines.
"""

import numpy as np

import concourse.bass as bass
import concourse.tile as tile
from concourse import bacc, mybir
from concourse.bass_utils import run_bass_kernel_spmd

N_NEURONS = 50000
N_BKG = 100          # K (contraction dim)
R = 5                # synapse basis
BT = 250             # batch*time
N_CORES = 8
NLOC = N_NEURONS // N_CORES       # 6250 rows per core
NR = NLOC * R                     # 31250 free-dim elements per core
BH = BT // 2                      # 125 real rows per half
BP = 128                          # padded partitions per half (16-engine DMA)

F32 = mybir.dt.float32
F32R = mybir.dt.float32r
I16 = mybir.dt.int16
TAU_SCALE = 32768.0

# fp32r matmul requires even free-dim counts and even element offsets;
# 5000-wide chunks split into 500-wide tiles (tail chunk 1250 = 2x500+250).
_CWS = [1250, 3750, 5000, 5000, 5000, 5000, 5000, 1250]
CHUNKS = []
_s = 0
for _cw in _CWS:
    CHUNKS.append((_s, _cw))
    _s += _cw
assert _s == NR


def _mm_tiles(width):
    """Split a chunk width into matmul free-dim tiles (<=500, f32 PSUM bank)."""
    tiles = []
    off = 0
    while off < width:
        tw = min(500, width - off)
        tiles.append((off, tw))
        off += tw
    return tiles


def _build_program(use_f32r=True):
    wdt = F32R if use_f32r else F32
    nc = bacc.Bacc("TRN2", target_bir_lowering=False, debug=False,
                   num_devices=N_CORES)

    w0_d = nc.dram_tensor("w0", [N_BKG, NLOC], F32, kind="ExternalInput").ap()
    tau0_d = nc.dram_tensor("tau0", [N_BKG, NR], I16, kind="ExternalInput").ap()
    w1_d = nc.dram_tensor("w1", [N_BKG, NLOC], F32, kind="ExternalInput").ap()
    tau1_d = nc.dram_tensor("tau1", [N_BKG, NR], I16, kind="ExternalInput").ap()
    spikesT_d = nc.dram_tensor("spikesT", [N_BKG, 2 * BP], F32,
                               kind="ExternalInput").ap()
    out_d = nc.dram_tensor("out", [2 * BP, NR], F32, kind="ExternalOutput").ap()

    with tile.TileContext(nc) as tc:
        with (
            tc.tile_pool(name="const", bufs=1) as const,
            tc.tile_pool(name="win", bufs=3) as win,
            tc.tile_pool(name="tauin", bufs=2) as tauin,
            tc.tile_pool(name="wbuild", bufs=2) as wbuild,
            tc.tile_pool(name="psum", bufs=8, space="PSUM") as psum,
            tc.tile_pool(name="stage", bufs=3) as stage,
        ):
            st_f32 = const.tile([N_BKG, 2 * BP], F32, tag="st_raw")
            nc.gpsimd.dma_start(st_f32[:], spikesT_d[:])
            st = const.tile([N_BKG, 2 * BP], wdt, tag="st")
            nc.vector.tensor_copy(st[:], st_f32[:])

            copy_i = 0
            for c, (s, cw) in enumerate(CHUNKS):
                ns, nw = s // R, cw // R
                w0c = win.tile([N_BKG, nw], F32, tag="w0c")
                nc.gpsimd.dma_start(w0c[:], w0_d[:, ns:ns + nw])
                w1c = win.tile([N_BKG, nw], F32, tag="w1c")
                nc.gpsimd.dma_start(w1c[:], w1_d[:, ns:ns + nw])
                tau0c = tauin.tile([N_BKG, cw], I16, tag="tau0c")
                # split 100 partitions as 64+36 so HWDGE spreads 16+12 engines
                nc.scalar.dma_start(tau0c[:64, :], tau0_d[:64, s:s + cw])
                nc.scalar.dma_start(tau0c[64:, :], tau0_d[64:, s:s + cw])
                tau1c = tauin.tile([N_BKG, cw], I16, tag="tau1c")
                nc.gpsimd.dma_start(tau1c[:], tau1_d[:, s:s + cw])

                W0c = wbuild.tile([N_BKG, cw], wdt, tag="W0c")
                nc.vector.tensor_mul(
                    W0c[:].rearrange("k (n r) -> k n r", r=R),
                    w0c[:].unsqueeze(2).broadcast_to([N_BKG, nw, R]),
                    tau0c[:].rearrange("k (n r) -> k n r", r=R),
                )
                W1c = wbuild.tile([N_BKG, cw], wdt, tag="W1c")
                nc.vector.tensor_mul(
                    W1c[:].rearrange("k (n r) -> k n r", r=R),
                    w1c[:].unsqueeze(2).broadcast_to([N_BKG, nw, R]),
                    tau1c[:].rearrange("k (n r) -> k n r", r=R),
                )

                for h in range(2):
                    st_h = st[:, h * BP:(h + 1) * BP]
                    stg = stage.tile([BP, cw], F32, tag="stage")
                    for (t0, tw) in _mm_tiles(cw):
                        ps = psum.tile([BP, tw], F32, tag="ps")
                        nc.tensor.matmul(ps[:], st_h, W0c[:, t0:t0 + tw],
                                         start=True, stop=False)
                        nc.tensor.matmul(ps[:], st_h, W1c[:, t0:t0 + tw],
                                         start=False, stop=True)
                        # alternate PSUM drains between ACT and DVE
                        if copy_i % 3 == 2:
                            nc.vector.tensor_copy(stg[:, t0:t0 + tw], ps[:])
                        else:
                            nc.scalar.copy(stg[:, t0:t0 + tw], ps[:])
                        copy_i += 1
                    nc.sync.dma_start(
                        out_d[h * BP:(h + 1) * BP, s:s + cw], stg[:])

    nc.compile()
    return nc


def _preprocess(weights, tau_syn, rows, cols):
    """Host index preprocessing: build per-core dense round arrays."""
    E = rows.shape[0]
    rows = rows.astype(np.int64)
    cols = cols.astype(np.int64)
    core = rows // NLOC
    nloc = rows % NLOC

    # bucket identity and slot (occurrence index) per edge
    q = rows * N_BKG + cols
    order = np.argsort(q, kind="stable")
    qs = q[order]
    first = np.r_[True, qs[1:] != qs[:-1]]
    run_id = np.cumsum(first) - 1
    run_starts = np.flatnonzero(first)
    run_len = np.diff(np.r_[run_starts, E])
    slot = np.empty(E, np.int64)
    slot[order] = np.arange(E) - run_starts[run_id]
    cnt = np.empty(E, np.int64)
    cnt[order] = run_len[run_id]

    # dense layouts: w [k, n] flat = k*NLOC + n ; tau [k, n*R+r]
    wflat = cols * NLOC + nloc
    tauflat = wflat * R  # == k*NR + n*R

    # tau arrays are sent as int16 fixed point: tau_i16 = round(tau*32768)
    # (clipped to 32767), and the matching w is pre-divided by 32768 (exact
    # power-of-2 scaling), so W = w_sent * int(tau_i16) on device.
    w0d = np.zeros((N_CORES, N_BKG * NLOC), np.float32)
    tau0d = np.zeros((N_CORES, N_BKG * NR), np.int16)
    w1d = np.zeros((N_CORES, N_BKG * NLOC), np.float32)
    tau1d = np.zeros((N_CORES, N_BKG * NR), np.int16)

    roff = np.arange(R)

    def q16(x):
        return np.minimum(np.rint(x * TAU_SCALE), 32767.0).astype(np.int16)

    m0 = slot == 0
    w0d[core[m0], wflat[m0]] = weights[m0] / TAU_SCALE
    tau0d[core[m0][:, None], tauflat[m0][:, None] + roff] = q16(tau_syn[m0])

    # round 1, buckets with exactly 2 edges: plain (w, tau)
    m1s = (slot == 1) & (cnt == 2)
    w1d[core[m1s], wflat[m1s]] = weights[m1s] / TAU_SCALE
    tau1d[core[m1s][:, None], tauflat[m1s][:, None] + roff] = q16(tau_syn[m1s])

    # buckets with >=3 edges (~0.2% of edges): merge slots>=1 on host
    # (w1 = pow2 scale / 32768, tau1 = round(sum_j>=1 w_j*tau_j / scale))
    mm = (slot >= 1) & (cnt >= 3)
    if mm.any():
        m1m = (slot == 1) & (cnt >= 3)
        vals1 = np.zeros((N_CORES, N_BKG * NR), np.float32)
        contrib = weights[mm, None] * tau_syn[mm]
        np.add.at(vals1, (core[mm][:, None], tauflat[mm][:, None] + roff),
                  contrib)
        ci, fi = core[m1m], wflat[m1m]
        v = vals1[ci[:, None], (fi * R)[:, None] + roff]      # [M, R]
        vmax = np.abs(v).max(axis=1)
        vmax = np.maximum(vmax, 1e-30)
        scale = np.exp2(np.ceil(np.log2(vmax)))
        w1d[ci, fi] = (scale / TAU_SCALE).astype(np.float32)
        qv = np.clip(np.rint(v / scale[:, None] * TAU_SCALE),
                     -32768.0, 32767.0).astype(np.int16)
        tau1d[ci[:, None], (fi * R)[:, None] + roff] = qv

    return (w0d.reshape(N_CORES, N_BKG, NLOC),
            tau0d.reshape(N_CORES, N_BKG, NR),
            w1d.reshape(N_CORES, N_BKG, NLOC),
            tau1d.reshape(N_CORES, N_BKG, NR))


_program_cache = {}


def get_program(use_f32r=True):
    key = bool(use_f32r)
    if key not in _program_cache:
        _program_cache[key] = _build_program(use_f32r)
    return _program_cache[key]


def make_in_maps(weights, tau_syn, spikes, rows, cols):
    weights = np.ascontiguousarray(np.asarray(weights, dtype=np.float32))
    tau_syn = np.ascontiguousarray(np.asarray(tau_syn, dtype=np.float32))
    spikes = np.ascontiguousarray(np.asarray(spikes, dtype=np.float32))
    rows = np.asarray(rows)
    cols = np.asarray(cols)

    w0d, tau0d, w1d, tau1d = _preprocess(weights, tau_syn, rows, cols)
    # pad spikesT columns to 2*BP=256: [0:125]=half0, [128:253]=half1
    spikesT = np.zeros((N_BKG, 2 * BP), np.float32)
    spikesT[:, 0:BH] = spikes.T[:, 0:BH]
    spikesT[:, BP:BP + BH] = spikes.T[:, BH:BT]

    in_maps = []
    for c in range(N_CORES):
        in_maps.append({
            "w0": np.ascontiguousarray(w0d[c]),
            "tau0": np.ascontiguousarray(tau0d[c]),
            "w1": np.ascontiguousarray(w1d[c]),
            "tau1": np.ascontiguousarray(tau1d[c]),
            "spikesT": spikesT,
        })
    return in_maps


def kernel(weights, tau_syn, spikes, rows, cols):
    nc = get_program(use_f32r=True)
    in_maps = make_in_maps(weights, tau_syn, spikes, rows, cols)
    res = run_bass_kernel_spmd(nc, in_maps, list(range(N_CORES)))
    full = np.concatenate(
        [np.concatenate([res.results[c]["out"][0:BH],
                         res.results[c]["out"][BP:BP + BH]], axis=0)
         for c in range(N_CORES)], axis=1)
    return full.reshape(1, BT, N_NEURONS * R)
